# revision 1
# baseline (speedup 1.0000x reference)
"""Trainium2 Bass kernel for local (block-sparse) scaled-dot-product attention.

Contract: kernel(**inputs) takes the FULL inputs of the reference
(query/key_in/value [8, 4096, 512] fp32, Wq/Wk/Wv/Wo [512, 512], biases [512])
and returns the FULL output [8, 4096, 512] fp32.

Sharding: data-parallel over batch; batch element b runs on NeuronCore b.

On-chip layout is feature-major ("transposed"): activations live as [feat, t]
so the contraction dim of every matmul is on partitions. The CPU pre-transposes
the inputs/weights (free) and transposes the output back.
"""

import math

import numpy as np
import ml_dtypes

import concourse.bass as bass
import concourse.tile as tile
from concourse import bacc, mybir
from concourse.bass_utils import run_bass_kernel_spmd

# ---- problem constants (hardcoded; must match the reference) ----
B, T, F = 8, 4096, 512
H, DK, DV = 8, 64, 64
CTX = 64          # block size (cq == ck == 64, nb == 64)
NB = T // CTX     # 64 blocks
NEG = -1e20
SCALE = 1.0 / math.sqrt(DK)

TG = 8            # t-groups per core
TT = T // TG      # 512 t positions per group
NB8 = TT // CTX   # 8 blocks per group

# bf16 everywhere on the matmul path (fp32 PSUM accumulation).
DT = mybir.dt.bfloat16
NP_DT = ml_dtypes.bfloat16
F32 = mybir.dt.float32

_CACHED = None  # (nc,) built once


def _flat(ap):
    # [p, a, b] -> [p, a*b] view of a contiguous tile
    return ap.rearrange("p a b -> p (a b)")


def _build_masks():
    """Rank-2 additive masks for the shifted-window attention.

    Scores tile sT[k, q] per block: k in window [64n-32, 64n+96) (j = 0..128),
    q = 0..64.  Invalid pairs get NEG via sum of 2 outer products mj[r] x mi[r].
      mid   : invalid = (j>=96 & q<32) | (j<32 & q>=32)
      first : invalid = (j<32, all q) | (j>=96 & q<32)      (block 0: no prev)
      last  : invalid = (j<32 & q>=32) | (j>=96, all q)     (block 63: no next)
    """
    j = np.arange(128)
    r = np.arange(64)
    mj = np.zeros((3, 2, 128), np.float32)
    mi = np.zeros((3, 2, 64), np.float32)
    # mid
    mj[0, 0] = NEG * (j >= 96)
    mi[0, 0] = (r < 32).astype(np.float32)
    mj[0, 1] = NEG * (j < 32)
    mi[0, 1] = (r >= 32).astype(np.float32)
    # first
    mj[1, 0] = NEG * (j < 32)
    mi[1, 0] = 1.0
    mj[1, 1] = NEG * (j >= 96)
    mi[1, 1] = (r < 32).astype(np.float32)
    # last
    mj[2, 0] = NEG * (j >= 96)
    mi[2, 0] = 1.0
    mj[2, 1] = NEG * (j < 32)
    mi[2, 1] = (r >= 32).astype(np.float32)
    return mj, mi


def _sumsel():
    # ss[p, 8*n8 + m] = 1 if m == n8 (lhsT for per-block column sums)
    s = np.zeros((128, 64), np.float32)
    for n8 in range(8):
        s[:, 8 * n8 + n8] = 1.0
    return s


def _rowsel():
    # rsel[m, 128*n8 + p] = 1 if m == n8 (lhsT to broadcast rs row n8)
    r = np.zeros((8, 1024), np.float32)
    for n8 in range(8):
        r[n8, 128 * n8 : 128 * n8 + 128] = 1.0
    return r


def _build_nc(n_iter=1):
    nc = bacc.Bacc(None, target_bir_lowering=False, debug=False)

    xq = nc.dram_tensor("xq", [F, T], DT, kind="ExternalInput")
    xk = nc.dram_tensor("xk", [F, T], DT, kind="ExternalInput")
    xv = nc.dram_tensor("xv", [F, T], DT, kind="ExternalInput")
    wq = nc.dram_tensor("wq", [F, F], DT, kind="ExternalInput")  # Wq.T
    wk = nc.dram_tensor("wk", [F, F], DT, kind="ExternalInput")  # Wk.T
    wv = nc.dram_tensor("wv", [F, F], DT, kind="ExternalInput")  # Wv.T
    wo = nc.dram_tensor("wo", [F, F], DT, kind="ExternalInput")  # Wo.T
    bq = nc.dram_tensor("bq", [F], F32, kind="ExternalInput")
    bk = nc.dram_tensor("bk", [F], F32, kind="ExternalInput")
    bv = nc.dram_tensor("bv", [F], F32, kind="ExternalInput")
    bo = nc.dram_tensor("bo", [F], F32, kind="ExternalInput")
    mj = nc.dram_tensor("mj", [3, 2, 128], DT, kind="ExternalInput")
    mi = nc.dram_tensor("mi", [3, 2, 64], DT, kind="ExternalInput")
    ss = nc.dram_tensor("ss", [128, 64], DT, kind="ExternalInput")
    rsel = nc.dram_tensor("rsel", [8, 1024], DT, kind="ExternalInput")
    outd = nc.dram_tensor("out", [F, T], F32, kind="ExternalOutput")

    Exp = mybir.ActivationFunctionType.Exp

    with tile.TileContext(nc) as tc:
        with (
            tc.tile_pool(name="singles", bufs=1) as singles,
            tc.tile_pool(name="xin", bufs=2) as xin,
            tc.tile_pool(name="proj_out", bufs=2) as pqk,
            tc.tile_pool(name="vpool", bufs=2) as vpool,
            tc.tile_pool(name="epool", bufs=3) as epool,
            tc.tile_pool(name="ypool", bufs=2) as ypool,
            tc.tile_pool(name="opool", bufs=2) as opool,
            tc.tile_pool(name="ps_proj", bufs=2, space="PSUM") as ps_proj,
            tc.tile_pool(name="ps_s", bufs=2, space="PSUM") as ps_s,
            tc.tile_pool(name="ps_r", bufs=2, space="PSUM") as ps_r,
            tc.tile_pool(name="ps_o", bufs=2, space="PSUM") as ps_o,
        ):
            # ---- static tiles ----
            wq_t = singles.tile([128, 4, F], DT, tag="wq")
            wk_t = singles.tile([128, 4, F], DT, tag="wk")
            wv_t = singles.tile([128, 4, F], DT, tag="wv")
            wo_t = singles.tile([128, 4, F], DT, tag="wo")
            for wt, wd in ((wq_t, wq), (wk_t, wk), (wv_t, wv), (wo_t, wo)):
                nc.sync.dma_start(out=wt, in_=wd.rearrange("(c p) o -> p c o", p=128))
            bq_t = singles.tile([128, 4], F32, tag="bq")
            bk_t = singles.tile([128, 4], F32, tag="bk")
            bv_t = singles.tile([128, 4], F32, tag="bv")
            bo_t = singles.tile([128, 4], F32, tag="bo")
            for bt, bd in ((bq_t, bq), (bk_t, bk), (bv_t, bv), (bo_t, bo)):
                nc.sync.dma_start(out=bt, in_=bd.rearrange("(c p) -> p c", p=128))
            mj_t = singles.tile([2, 3, 128], DT, tag="mj")
            nc.sync.dma_start(out=mj_t, in_=mj.rearrange("k r j -> r k j"))
            mi_t = singles.tile([2, 3, 64], DT, tag="mi")
            nc.sync.dma_start(out=mi_t, in_=mi.rearrange("k r j -> r k j"))
            ss_t = singles.tile([128, 64], DT, tag="ss")
            nc.sync.dma_start(out=ss_t, in_=ss[:, :])
            rsel_t = singles.tile([8, 1024], DT, tag="rsel")
            nc.sync.dma_start(out=rsel_t, in_=rsel[:, :])

            xq_r = xq.rearrange("(c p) t -> p c t", p=128)
            xk_r = xk.rearrange("(c p) t -> p c t", p=128)
            xv_r = xv.rearrange("(c p) t -> p c t", p=128)
            out_r = outd.rearrange("(c p) t -> p c t", p=128)

            def emit_group(tg):
                t0 = tg * TT
                # ---- input loads ----
                xq_s = xin.tile([128, 4, TT], DT, tag="xq")
                nc.sync.dma_start(out=xq_s, in_=xq_r[:, :, t0 : t0 + TT])
                lo, hi = t0 - 32, t0 + TT + 32
                clo, chi = max(lo, 0), min(hi, T)
                xk_s = xin.tile([128, 4, TT + 64], DT, tag="xk")
                xv_s = xin.tile([128, 4, TT + 64], DT, tag="xv")
                for xs, xr in ((xk_s, xk_r), (xv_s, xv_r)):
                    nc.sync.dma_start(
                        out=xs[:, :, clo - lo : chi - lo], in_=xr[:, :, clo:chi]
                    )
                    if clo > lo:
                        nc.vector.memset(xs[:, :, 0 : clo - lo], 0.0)
                    if chi < hi:
                        nc.vector.memset(xs[:, :, TT + 64 - (hi - chi) :], 0.0)

                # ---- q/k projections (feature-major) ----
                qT = pqk.tile([128, 4, TT], DT, tag="qT")
                kT = pqk.tile([128, 4, TT + 64], DT, tag="kT")
                for oc in range(4):
                    ps = ps_proj.tile([128, 512], F32, tag="proj")
                    for fc in range(4):
                        nc.tensor.matmul(
                            ps,
                            lhsT=wq_t[:, fc, oc * 128 : (oc + 1) * 128],
                            rhs=xq_s[:, fc, :],
                            start=(fc == 0),
                            stop=(fc == 3),
                        )
                    nc.vector.tensor_scalar_add(qT[:, oc, :], ps, bq_t[:, oc : oc + 1])
                    ps = ps_proj.tile([128, 512], F32, tag="proj")
                    for fc in range(4):
                        nc.tensor.matmul(
                            ps,
                            lhsT=wk_t[:, fc, oc * 128 : (oc + 1) * 128],
                            rhs=xk_s[:, fc, 0:512],
                            start=(fc == 0),
                            stop=(fc == 3),
                        )
                    nc.vector.tensor_scalar_add(
                        kT[:, oc, 0:512], ps, bk_t[:, oc : oc + 1]
                    )
                    ps2 = ps_proj.tile([128, 64], F32, tag="proj")
                    for fc in range(4):
                        nc.tensor.matmul(
                            ps2,
                            lhsT=wk_t[:, fc, oc * 128 : (oc + 1) * 128],
                            rhs=xk_s[:, fc, 512:576],
                            start=(fc == 0),
                            stop=(fc == 3),
                        )
                    nc.vector.tensor_scalar_add(
                        kT[:, oc, 512:576], ps2, bk_t[:, oc : oc + 1]
                    )

                # ---- v projection (t-major), covering [t0-32, t0+544) ----
                v0 = vpool.tile([128, 5, F], DT, tag="v0")
                for tc5 in range(5):
                    m = 128 if tc5 < 4 else 64
                    ps = ps_proj.tile([128, 512], F32, tag="proj")
                    for fc in range(4):
                        nc.tensor.matmul(
                            ps[0:m, :],
                            lhsT=xv_s[:, fc, 128 * tc5 : 128 * tc5 + m],
                            rhs=wv_t[:, fc, :],
                            start=(fc == 0),
                            stop=(fc == 3),
                        )
                    nc.vector.tensor_copy(out=v0[0:m, tc5, :], in_=ps[0:m, :])
                # shifted copy: v0s covers [t0+32, t0+544), chunk c = rows
                # [64..128) of v0 chunk c plus rows [0..64) of v0 chunk c+1.
                v0s = vpool.tile([128, 4, F], DT, tag="v0s")
                nc.sync.dma_start(out=v0s[0:64, :, :], in_=v0[64:128, 0:4, :])
                nc.sync.dma_start(out=v0s[64:128, :, :], in_=v0[0:64, 1:5, :])

                # ---- attention, per head over 8 blocks ----
                yT = ypool.tile([128, 4, TT], DT, tag="yT")
                for hp in range(H // 2):
                    oc = hp
                    # Emit both heads' QK matmuls adjacently: they occupy
                    # disjoint 32-row groups (partitions 0-63 vs 64-127), so
                    # back-to-back MMs pack concurrently in the PE array.
                    sT0 = ps_s.tile([128, NB8, 64], F32, tag="sT")
                    sT1 = ps_s.tile([128, NB8, 64], F32, tag="sT")
                    sTs = (sT0, sT1)
                    for n8 in range(NB8):
                        n = tg * NB8 + n8
                        kind = 1 if n == 0 else (2 if n == NB - 1 else 0)
                        for hl in range(2):
                            pb = hl * 64
                            nc.tensor.matmul(
                                sTs[hl][:, n8, :],
                                lhsT=kT[pb : pb + 64, oc, 64 * n8 : 64 * n8 + 128],
                                rhs=qT[pb : pb + 64, oc, 64 * n8 : 64 * n8 + 64],
                                start=True,
                                stop=False,
                            )
                        for hl in range(2):
                            nc.tensor.matmul(
                                sTs[hl][:, n8, :],
                                lhsT=mj_t[:, kind, :],
                                rhs=mi_t[:, kind, :],
                                start=False,
                                stop=True,
                            )
                    oT = None
                    for hl in range(2):
                        h = 2 * hp + hl
                        pb = hl * 64
                        sT = sTs[hl]
                        eT = epool.tile([128, NB8, 64], DT, tag="eT")
                        nc.scalar.activation(out=eT, in_=sT, func=Exp, scale=SCALE)
                        sums = ps_r.tile([8, 64], F32, tag="r")
                        for n8 in range(NB8):
                            nc.tensor.matmul(
                                sums,
                                lhsT=ss_t[:, 8 * n8 : 8 * n8 + 8],
                                rhs=eT[:, n8, :],
                                start=(n8 == 0),
                                stop=(n8 == NB8 - 1),
                            )
                        rs = epool.tile([8, 64], DT, tag="rs")
                        with nc.allow_low_precision(reason="bf16 softmax denominators"):
                            nc.vector.reciprocal(out=rs, in_=sums)
                        bc = ps_r.tile([128, NB8, 64], F32, tag="r")
                        for n8 in range(NB8):
                            nc.tensor.matmul(
                                bc[:, n8, :],
                                lhsT=rsel_t[:, 128 * n8 : 128 * n8 + 128],
                                rhs=rs,
                                start=True,
                                stop=True,
                            )
                        eN = epool.tile([128, NB8, 64], DT, tag="eN")
                        nc.vector.tensor_mul(_flat(eN), _flat(eT), _flat(bc))
                        if pb == 0:
                            oT = ps_o.tile([128, 512], F32, tag="oT")
                        for n8 in range(NB8):
                            if n8 % 2 == 0:
                                lhsT = v0[:, n8 // 2, 64 * h : 64 * h + 64]
                            else:
                                lhsT = v0s[:, (n8 - 1) // 2, 64 * h : 64 * h + 64]
                            nc.tensor.matmul(
                                oT[pb : pb + 64, 64 * n8 : 64 * n8 + 64],
                                lhsT=lhsT,
                                rhs=eN[:, n8, :],
                                start=True,
                                stop=True,
                                tile_position=(0, pb),
                            )
                        if pb == 64:
                            nc.vector.tensor_scalar_add(
                                yT[:, oc, :], oT, bv_t[:, oc : oc + 1]
                            )

                # ---- output projection ----
                outsb = opool.tile([128, 4, TT], F32, tag="outsb")
                for oc in range(4):
                    ps = ps_proj.tile([128, 512], F32, tag="proj")
                    for fc in range(4):
                        nc.tensor.matmul(
                            ps,
                            lhsT=wo_t[:, fc, oc * 128 : (oc + 1) * 128],
                            rhs=yT[:, fc, :],
                            start=(fc == 0),
                            stop=(fc == 3),
                        )
                    nc.vector.tensor_scalar_add(
                        outsb[:, oc, :], ps, bo_t[:, oc : oc + 1]
                    )
                nc.sync.dma_start(out=out_r[:, :, t0 : t0 + TT], in_=outsb)

            if n_iter == 1:
                for tg in range(TG):
                    emit_group(tg)
            else:
                with tc.For_i(0, n_iter, 1):
                    for tg in range(TG):
                        emit_group(tg)

    nc.finalize()
    return nc


def _get_nc(n_iter=1):
    global _CACHED
    if _CACHED is None:
        _CACHED = {}
    if n_iter not in _CACHED:
        _CACHED[n_iter] = _build_nc(n_iter)
    return _CACHED[n_iter]


def _prep_in_maps(query, key_in, value, Wq, bq, Wk, bk, Wv, bv, Wo, bo):
    mj, mi = _build_masks()
    shared = {
        "wq": np.ascontiguousarray(Wq.T).astype(NP_DT),
        "wk": np.ascontiguousarray(Wk.T).astype(NP_DT),
        "wv": np.ascontiguousarray(Wv.T).astype(NP_DT),
        "wo": np.ascontiguousarray(Wo.T).astype(NP_DT),
        "bq": np.asarray(bq, np.float32),
        "bk": np.asarray(bk, np.float32),
        "bv": np.asarray(bv, np.float32),
        "bo": np.asarray(bo, np.float32),
        "mj": mj.astype(NP_DT),
        "mi": mi.astype(NP_DT),
        "ss": _sumsel().astype(NP_DT),
        "rsel": _rowsel().astype(NP_DT),
    }
    from concurrent.futures import ThreadPoolExecutor

    def _tp(a):
        return np.ascontiguousarray(np.asarray(a, np.float32).T.astype(NP_DT))

    with ThreadPoolExecutor(12) as ex:
        xqs = list(ex.map(_tp, [query[b] for b in range(B)]))
        xks = list(ex.map(_tp, [key_in[b] for b in range(B)]))
        xvs = list(ex.map(_tp, [value[b] for b in range(B)]))
    in_maps = []
    for b in range(B):
        in_maps.append({"xq": xqs[b], "xk": xks[b], "xv": xvs[b], **shared})
    return in_maps


def run(trace=False, **inputs):
    nc = _get_nc()
    in_maps = _prep_in_maps(**inputs)
    res = run_bass_kernel_spmd(
        nc, in_maps, core_ids=list(range(B)), trace=trace
    )
    out = np.stack(
        [np.asarray(res.results[b]["out"], np.float32).T for b in range(B)]
    )
    return out, res


def kernel(**inputs):
    out, _ = run(trace=False, **inputs)
    return out



# revision 17
# speedup vs baseline: 158.2796x; 158.2796x over previous
"""Trainium2 Bass kernel for local (block-sparse) scaled-dot-product attention.

Contract: kernel(**inputs) takes the FULL inputs of the reference
(query/key_in/value [8, 4096, 512] fp32, Wq/Wk/Wv/Wo [512, 512], biases [512])
and returns the FULL output [8, 4096, 512] fp32.

Sharding: data-parallel over batch; batch element b runs on NeuronCore b.

On-chip layout is feature-major ("transposed"): activations live as [feat, t]
so the contraction dim of every matmul is on partitions. The CPU pre-transposes
the inputs/weights (free) and transposes the output back.

Numerics / algebra:
  - bk is dropped entirely: a per-query constant in the logits is
    softmax-invariant.
  - bv is folded into the output bias host-side (bo' = bo + Wo @ bv), since
    softmax weights sum to 1.
  - q/k projections run in fp8 (e4m3) with DoubleRow perf mode; the weight is
    pre-scaled by 16 to stay in e4m3's normal range and the 1/16 is undone in
    the PSUM->SBUF copy. fp8 noise only perturbs logits by ~1e-3 relative.
  - v/o projections and all attention matmuls stay bf16 (fp32 PSUM).
  - Softmax: scores land k-major ([k_window, q]); the shifted-window mask is
    added by ONE rank-4 matmul per head over the whole 8-block tile; the
    denominator is computed broadcast across partitions by a ones-matmul, so
    normalization is a single elementwise multiply that doubles as the
    PSUM->SBUF copy of the attention output.
"""

import math
import os

import numpy as np
import ml_dtypes

import concourse.bass as bass
import concourse.tile as tile
from concourse import bacc, mybir
from concourse.bass_utils import run_bass_kernel_spmd

# ---- problem constants (hardcoded; must match the reference) ----
B, T, F = 8, 4096, 512
H, DK, DV = 8, 64, 64
CTX = 64          # block size (cq == ck == 64, nb == 64)
NB = T // CTX     # 64 blocks
NEG = -1e20
SCALE = 1.0 / math.sqrt(DK)
WSCALE = 16.0     # fp8 weight pre-scale (undone in the PSUM->SBUF copy)

TG = 8            # t-groups per core
TT = T // TG      # 512 t positions per group
NB8 = TT // CTX   # 8 blocks per group

DT = mybir.dt.bfloat16
NP_DT = ml_dtypes.bfloat16
USE_FP8 = os.environ.get("K_FP8", "1") == "1"
F8 = mybir.dt.float8e4 if USE_FP8 else mybir.dt.bfloat16
NP_F8 = ml_dtypes.float8_e4m3 if USE_FP8 else ml_dtypes.bfloat16
F32 = mybir.dt.float32
QK_CHUNKS = 2 if USE_FP8 else 4
QK_PERF = mybir.MatmulPerfMode.DoubleRow if USE_FP8 else None

_CACHED = None


def _flat(ap):
    # [p, a, b] -> [p, a*b] view of a contiguous tile
    return ap.rearrange("p a b -> p (a b)")


def _build_masks():
    """Rank-4 additive mask for the shifted-window attention.

    Scores tile sT[k, (n8, q)]: k in window [64n-32, 64n+96) (j = 0..128),
    q = 0..64, for the 8 blocks n8 of one t-group.  Invalid pairs get NEG via
    mja.T @ mia (a K=4 matmul accumulated into the scores PSUM):
      row0: NEG*(j>=96) x (q<32)            -- mid mask, both halves
      row1: NEG*(j<32)  x (q>=32)
      row2: NEG*(j<32)  x (q<32)*(n8==0)    -- extra for global block 0
      row3: NEG*(j>=96) x (q>=32)*(n8==7)   -- extra for global block 63
    mia comes in 3 variants: 0=mid groups, 1=first group (tg 0), 2=last (tg 7).
    """
    j = np.arange(128)
    mja = np.zeros((4, 128), np.float32)
    mja[0] = NEG * (j >= 96)
    mja[1] = NEG * (j < 32)
    mja[2] = NEG * (j < 32)
    mja[3] = NEG * (j >= 96)

    q = np.arange(512) % 64
    n8 = np.arange(512) // 64
    mia = np.zeros((3, 4, 512), np.float32)
    for v in range(3):
        mia[v, 0] = q < 32
        mia[v, 1] = q >= 32
    mia[1, 2] = (q < 32) & (n8 == 0)
    mia[2, 3] = (q >= 32) & (n8 == 7)
    return mja, mia


def _build_nc(n_iter=1):
    nc = bacc.Bacc(None, target_bir_lowering=False, debug=False)

    xq = nc.dram_tensor("xq", [F, T], F8, kind="ExternalInput")
    xk = nc.dram_tensor("xk", [F, T], F8, kind="ExternalInput")
    xv = nc.dram_tensor("xv", [F, T], DT, kind="ExternalInput")
    wq = nc.dram_tensor("wq", [F, F], F8, kind="ExternalInput")  # 16*Wq.T
    wk = nc.dram_tensor("wk", [F, F], F8, kind="ExternalInput")  # 16*Wk.T
    wv = nc.dram_tensor("wv", [F, F], DT, kind="ExternalInput")  # Wv.T
    wo = nc.dram_tensor("wo", [F, F], DT, kind="ExternalInput")  # Wo.T
    bq = nc.dram_tensor("bq", [F], F32, kind="ExternalInput")
    bo = nc.dram_tensor("bo", [F], F32, kind="ExternalInput")    # bo + Wo@bv
    mja = nc.dram_tensor("mja", [4, 128], DT, kind="ExternalInput")
    mia = nc.dram_tensor("mia", [3, 4, 512], DT, kind="ExternalInput")
    outd = nc.dram_tensor("out", [F, T], F32, kind="ExternalOutput")

    Exp = mybir.ActivationFunctionType.Exp
    Ident = mybir.ActivationFunctionType.Identity
    Copy = mybir.ActivationFunctionType.Copy

    with tile.TileContext(nc) as tc:
        with (
            tc.tile_pool(name="singles", bufs=1) as singles,
            tc.tile_pool(name="xin", bufs=2) as xin,
            tc.tile_pool(name="proj_out", bufs=2) as pqk,
            tc.tile_pool(name="vpool", bufs=2) as vpool,
            tc.tile_pool(name="epool", bufs=4) as epool,
            tc.tile_pool(name="rpool", bufs=2) as rpool,
            tc.tile_pool(name="ypool", bufs=2) as ypool,
            tc.tile_pool(name="opool", bufs=2) as opool,
            tc.tile_pool(name="ps_proj", bufs=2, space="PSUM") as ps_proj,
            tc.tile_pool(name="ps_s", bufs=1, space="PSUM") as ps_s,
            tc.tile_pool(name="ps_S", bufs=2, space="PSUM") as ps_S,
            tc.tile_pool(name="ps_o", bufs=2, space="PSUM") as ps_o,
        ):
            # ---- static tiles ----
            wq_t = singles.tile([128, 4, F], F8, tag="wq")
            wk_t = singles.tile([128, 4, F], F8, tag="wk")
            wv_t = singles.tile([128, 4, F], DT, tag="wv")
            wo_t = singles.tile([128, 4, F], DT, tag="wo")
            for wt, wd in ((wq_t, wq), (wk_t, wk), (wv_t, wv), (wo_t, wo)):
                nc.sync.dma_start(out=wt, in_=wd.rearrange("(c p) o -> p c o", p=128))
            bq_t = singles.tile([128, 4], F32, tag="bq")
            bo_t = singles.tile([128, 4], F32, tag="bo")
            for bt, bd in ((bq_t, bq), (bo_t, bo)):
                nc.sync.dma_start(out=bt, in_=bd.rearrange("(c p) -> p c", p=128))
            mja_t = singles.tile([4, 128], DT, tag="mja")
            nc.sync.dma_start(out=mja_t, in_=mja[:, :])
            mia_t = singles.tile([4, 3, 512], DT, tag="mia")
            nc.sync.dma_start(out=mia_t, in_=mia.rearrange("v r j -> r v j"))
            ones_t = singles.tile([128, 64], DT, tag="ones")
            nc.vector.memset(ones_t, 1.0)

            xq_r = xq.rearrange("(c p) t -> p c t", p=128)
            xk_r = xk.rearrange("(c p) t -> p c t", p=128)
            xv_r = xv.rearrange("(c p) t -> p c t", p=128)
            out_r = outd.rearrange("(c p) t -> p c t", p=128)

            def emit_loads(tg):
                t0 = tg * TT
                xq_s = xin.tile([128, 4, TT], F8, tag="xq")
                nc.sync.dma_start(out=xq_s, in_=xq_r[:, :, t0 : t0 + TT])
                lo, hi = t0 - 32, t0 + TT + 32
                clo, chi = max(lo, 0), min(hi, T)
                xk_s = xin.tile([128, 4, TT + 64], F8, tag="xk")
                xv_s = xin.tile([128, 4, TT + 64], DT, tag="xv")
                for xs, xr in ((xk_s, xk_r), (xv_s, xv_r)):
                    nc.sync.dma_start(
                        out=xs[:, :, clo - lo : chi - lo], in_=xr[:, :, clo:chi]
                    )
                    if clo > lo:
                        nc.vector.memset(xs[:, :, 0 : clo - lo], 0.0)
                    if chi < hi:
                        nc.vector.memset(xs[:, :, TT + 64 - (hi - chi) :], 0.0)
                return xq_s, xk_s, xv_s

            def emit_projqkv(tg, loads):
                xq_s, xk_s, xv_s = loads
                # ---- q/k projections (feature-major, fp8 DoubleRow) ----
                qT = pqk.tile([128, 4, TT], DT, tag="qT")
                kT = pqk.tile([128, 4, TT + 64], DT, tag="kT")
                nch = 4 // QK_CHUNKS   # k-tiles fused per matmul
                for oc in range(4):
                    ps = ps_proj.tile([128, 512], F32, tag="proj")
                    for j in range(QK_CHUNKS):
                        nc.tensor.matmul(
                            ps,
                            lhsT=wk_t[:, nch * j : nch * (j + 1), oc * 128 : (oc + 1) * 128],
                            rhs=xk_s[:, nch * j : nch * (j + 1), 0:512],
                            start=(j == 0),
                            stop=(j == QK_CHUNKS - 1),
                            perf_mode=QK_PERF,
                        )
                    nc.scalar.activation(
                        out=kT[:, oc, 0:512], in_=ps, func=Copy, scale=1.0 / WSCALE
                    )
                    ps2 = ps_proj.tile([128, 64], F32, tag="proj")
                    for j in range(QK_CHUNKS):
                        nc.tensor.matmul(
                            ps2,
                            lhsT=wk_t[:, nch * j : nch * (j + 1), oc * 128 : (oc + 1) * 128],
                            rhs=xk_s[:, nch * j : nch * (j + 1), 512:576],
                            start=(j == 0),
                            stop=(j == QK_CHUNKS - 1),
                            perf_mode=QK_PERF,
                        )
                    nc.scalar.activation(
                        out=kT[:, oc, 512:576], in_=ps2, func=Copy, scale=1.0 / WSCALE
                    )
                    ps = ps_proj.tile([128, 512], F32, tag="proj")
                    for j in range(QK_CHUNKS):
                        nc.tensor.matmul(
                            ps,
                            lhsT=wq_t[:, nch * j : nch * (j + 1), oc * 128 : (oc + 1) * 128],
                            rhs=xq_s[:, nch * j : nch * (j + 1), :],
                            start=(j == 0),
                            stop=(j == QK_CHUNKS - 1),
                            perf_mode=QK_PERF,
                        )
                    nc.scalar.activation(
                        out=qT[:, oc, :], in_=ps, func=Ident,
                        bias=bq_t[:, oc : oc + 1], scale=1.0 / WSCALE,
                    )

                # ---- v projection (t-major), covering [t0-32, t0+544) ----
                v0 = vpool.tile([128, 5, F], DT, tag="v0")
                for tc5 in range(5):
                    m = 128 if tc5 < 4 else 64
                    ps = ps_proj.tile([128, 512], F32, tag="proj")
                    for fc in range(4):
                        nc.tensor.matmul(
                            ps[0:m, :],
                            lhsT=xv_s[:, fc, 128 * tc5 : 128 * tc5 + m],
                            rhs=wv_t[:, fc, :],
                            start=(fc == 0),
                            stop=(fc == 3),
                        )
                    nc.vector.tensor_copy(out=v0[0:m, tc5, :], in_=ps[0:m, :])
                # shifted copy: v0s covers [t0+32, t0+544), chunk c = rows
                # [64..128) of v0 chunk c plus rows [0..64) of v0 chunk c+1.
                v0s = vpool.tile([128, 4, F], DT, tag="v0s")
                nc.sync.dma_start(out=v0s[0:64, :, :], in_=v0[64:128, 0:4, :])
                nc.sync.dma_start(out=v0s[64:128, :, :], in_=v0[0:64, 1:5, :])
                return qT, kT, v0, v0s

            def emit_attn(tg, proj):
                qT, kT, v0, v0s = proj
                var = 1 if tg == 0 else (2 if tg == TG - 1 else 0)
                yT = ypool.tile([128, 4, TT], DT, tag="yT")

                def emit_scores(hp):
                    oc = hp
                    sT = ps_s.tile([128, 2, NB8, 64], F32, tag="sT")
                    # The mask matmul opens the accumulation group over each
                    # bank (start=True zeroes the 2KB zero-region); QK
                    # matmuls then accumulate into it, last one closes it.
                    for hl in range(2):
                        nc.tensor.matmul(
                            sT[:, hl].rearrange("p a b -> p (a b)"),
                            lhsT=mja_t,
                            rhs=mia_t[:, var, :],
                            start=True,
                            stop=False,
                        )
                    for n8 in range(NB8):
                        for hl in range(2):
                            pb = hl * 64
                            nc.tensor.matmul(
                                sT[:, hl, n8, :],
                                lhsT=kT[pb : pb + 64, oc, 64 * n8 : 64 * n8 + 128],
                                rhs=qT[pb : pb + 64, oc, 64 * n8 : 64 * n8 + 64],
                                start=False,
                                stop=(n8 == NB8 - 1),
                            )
                    eT = epool.tile([128, 2, NB8, 64], DT, tag="eT")
                    nc.scalar.activation(
                        out=eT.rearrange("p a b c -> p (a b c)"),
                        in_=sT.rearrange("p a b c -> p (a b c)"),
                        func=Exp, scale=SCALE,
                    )
                    return eT

                def emit_av(hp, eT):
                    oc = hp
                    # broadcast denominators: S[p, j] = sum_k eT_h(p)[k, j]
                    S = ps_S.tile([128, 512], F32, tag="S")
                    for hl in range(2):
                        pb = hl * 64
                        nc.tensor.matmul(
                            S[pb : pb + 64, :],
                            lhsT=ones_t,
                            rhs=eT[:, hl].rearrange("p a b -> p (a b)"),
                            start=True,
                            stop=True,
                            tile_position=(0, pb),
                        )
                    rS = rpool.tile([128, 512], DT, tag="rS")
                    with nc.allow_low_precision(reason="bf16 softmax denominators"):
                        nc.vector.reciprocal(out=rS, in_=S)
                    oT = ps_o.tile([128, 512], F32, tag="oT")
                    for hl in range(2):
                        pb = hl * 64
                        h = 2 * hp + hl
                        for n8 in range(NB8):
                            if n8 % 2 == 0:
                                lhsT = v0[:, n8 // 2, 64 * h : 64 * h + 64]
                            else:
                                lhsT = v0s[:, (n8 - 1) // 2, 64 * h : 64 * h + 64]
                            nc.tensor.matmul(
                                oT[pb : pb + 64, 64 * n8 : 64 * n8 + 64],
                                lhsT=lhsT,
                                rhs=eT[:, hl, n8, :],
                                start=True,
                                stop=True,
                                tile_position=(0, pb),
                            )
                    # normalize + PSUM->SBUF in one op
                    nc.vector.tensor_mul(yT[:, oc, :], oT, rS)

                # software pipeline: QK/mask of hp overlaps exp of hp-1 on ACT
                prev = None
                for hp in range(H // 2):
                    eT = emit_scores(hp)
                    if prev is not None:
                        emit_av(prev[0], prev[1])
                    prev = (hp, eT)
                emit_av(prev[0], prev[1])
                return yT

            def emit_oproj(tg, yT):
                outsb = opool.tile([128, 4, TT], F32, tag="outsb")
                for oc in range(4):
                    ps = ps_proj.tile([128, 512], F32, tag="proj")
                    for fc in range(4):
                        nc.tensor.matmul(
                            ps,
                            lhsT=wo_t[:, fc, oc * 128 : (oc + 1) * 128],
                            rhs=yT[:, fc, :],
                            start=(fc == 0),
                            stop=(fc == 3),
                        )
                    nc.vector.tensor_scalar_add(
                        outsb[:, oc, :], ps, bo_t[:, oc : oc + 1]
                    )
                return outsb

            def emit_store(tg, outsb):
                t0 = tg * TT
                nc.sync.dma_start(out=out_r[:, :, t0 : t0 + TT], in_=outsb)

            # Software-pipelined across groups AND loop iterations: input DMAs
            # for group g+1 (mod 8, crossing the For_i seam) issue on SP
            # before the output store of group g-1, and group 7's o-proj +
            # store run during the next iteration's group-0 projections.
            state = {"loads": None, "pend_y": None}

            def emit_body():
                for tg in range(TG):
                    proj = emit_projqkv(tg, state["loads"])
                    pend_out = None
                    if state["pend_y"] is not None:
                        ptg, pyT = state["pend_y"]
                        pend_out = (ptg, emit_oproj(ptg, pyT))
                    state["loads"] = emit_loads((tg + 1) % TG)
                    if pend_out is not None:
                        emit_store(pend_out[0], pend_out[1])
                    yT = emit_attn(tg, proj)
                    state["pend_y"] = (tg, yT)

            state["loads"] = emit_loads(0)
            if n_iter == 1:
                emit_body()
            elif n_iter < 0:
                # Python-unrolled repeats: TimelineSim can't run For_i
                # (register branches need an executor), so steady-state
                # marginal time is measured on an unrolled build.
                for _ in range(-n_iter):
                    emit_body()
            else:
                with tc.For_i(0, n_iter, 1):
                    emit_body()
            ptg, pyT = state["pend_y"]
            emit_store(ptg, emit_oproj(ptg, pyT))

    nc.finalize()
    return nc


def _get_nc(n_iter=1):
    global _CACHED
    if _CACHED is None:
        _CACHED = {}
    if n_iter not in _CACHED:
        _CACHED[n_iter] = _build_nc(n_iter)
    return _CACHED[n_iter]


def _prep_in_maps(query, key_in, value, Wq, bq, Wk, bk, Wv, bv, Wo, bo):
    mja, mia = _build_masks()
    Wq = np.asarray(Wq, np.float32)
    Wk = np.asarray(Wk, np.float32)
    Wv = np.asarray(Wv, np.float32)
    Wo = np.asarray(Wo, np.float32)
    shared = {
        "wq": np.ascontiguousarray((WSCALE * Wq).T).astype(NP_F8),
        "wk": np.ascontiguousarray((WSCALE * Wk).T).astype(NP_F8),
        "wv": np.ascontiguousarray(Wv.T).astype(NP_DT),
        "wo": np.ascontiguousarray(Wo.T).astype(NP_DT),
        "bq": np.asarray(bq, np.float32),
        "bo": np.asarray(bo, np.float32) + Wo @ np.asarray(bv, np.float32),
        "mja": mja.astype(NP_DT),
        "mia": mia.astype(NP_DT),
    }
    from concurrent.futures import ThreadPoolExecutor

    def _tp8(a):
        return np.ascontiguousarray(np.asarray(a, np.float32).T.astype(NP_F8))

    def _tp(a):
        return np.ascontiguousarray(np.asarray(a, np.float32).T.astype(NP_DT))

    with ThreadPoolExecutor(12) as ex:
        xqs = list(ex.map(_tp8, [query[b] for b in range(B)]))
        xks = list(ex.map(_tp8, [key_in[b] for b in range(B)]))
        xvs = list(ex.map(_tp, [value[b] for b in range(B)]))
    in_maps = []
    for b in range(B):
        in_maps.append({"xq": xqs[b], "xk": xks[b], "xv": xvs[b], **shared})
    return in_maps


def run(trace=False, **inputs):
    nc = _get_nc()
    in_maps = _prep_in_maps(**inputs)
    res = run_bass_kernel_spmd(
        nc, in_maps, core_ids=list(range(B)), trace=trace
    )
    out = np.stack(
        [np.asarray(res.results[b]["out"], np.float32).T for b in range(B)]
    )
    return out, res


def kernel(**inputs):
    out, _ = run(trace=False, **inputs)
    return out
